# revision 50
# baseline (speedup 1.0000x reference)
"""AxisAttention TRN2 Bass kernel.

Full-input contract: kernel(**inputs) takes the unsharded numpy inputs and
returns the full [4, 2048, 512] float32 output.

Sharding: data-parallel over (batch, query-half) -> 8 NeuronCores. Each core
computes attention for 1024 queries of one batch against that batch's full
2048 keys. Params are replicated. K/V projections are recomputed by the two
cores sharing a batch (cheaper than a cross-core exchange).

Math per core (n=1024 queries, m=2048 keys, d=a=c=512):
  qT[a,n]  = sum_d WqS[d,a] * xqT[d,n]          (WqS = Wq*sqrt(512), fp16)
  kT[a,m]  = sum_d Wk[d,a] * xkvT[d,m]          (fp16)
  v[m,c]   = sum_d xkv8[d,m] * Wv8[d,c] / 64    (fp8 DoubleRow, Wv8=Wv*64)
  S[n,m]   = sum_a qT[a,n] * kT[a,m]            (fp16, PSUM f32)
  P[n,m]   = exp(S - rowmax(S)) in fp8; rowsum via ACT accum (f32)
  PT[m,n]  = DMA-xbar transpose of P as packed uint16 key-pairs
  OT[c,n]  = sum_m v[m,c] * PT[m,n]             (fp8 DoubleRow)
  YT[dq,n] = sum_c Wo8[c,dq] * OT[c,n]          (fp8 DoubleRow, Wo8=Wo*64)
  out[n,:] = Y * (1/(64*rowsum))[n] + query32[n,:]  (+bo broadcast)

Precision split: the score path (q/k projections, S) stays fp16 because the
reference multiplies scores by sqrt(512) -> logits with std ~100, so softmax
is near-argmax and tiny score errors flip rows. The value path (v, P*v, out
projection) tolerates fp8e4m3; weights are pre-scaled by 64 so w~0.02 values
stay out of fp8 subnormal range, descaled in the f32 epilogue.

Scheduling: one PSUM pool region for everything (pool transitions insert
multi-us barriers). Tag "S" = 3 x [128,1024] score buffers (6 banks); tag
"ot" = 2 x [128,512] (2 banks) shared by all projection / PV / Y psums.
All loads issue from the sync queue in consumption order (staggered issue
keeps DMA descriptor streams from thrashing DRAM); exp -> half-transpose
is issued per score half so the last tiles' P^T chain stays short. The
P^T transpose moves packed uint16 byte-pairs of the fp8 P, which lands
adjacent keys in one partition's byte lanes -- exactly the interleaved
rhs layout the DoubleRow PV matmul consumes, so no separate cast pass
exists. PSUM->SBUF copies are split across the scalar and vector queues
so neither becomes a convoy for the softmax-tail exps that gate PV.
"""

import numpy as np
import ml_dtypes

import concourse.bass as bass
import concourse.mybir as mybir
import concourse.tile as tile
from concourse import bacc
from concourse.bass_utils import run_bass_kernel_spmd

F8 = mybir.dt.float8e4
F16 = mybir.dt.float16
F32 = mybir.dt.float32
AX = mybir.AxisListType
ALU = mybir.AluOpType
ACTF = mybir.ActivationFunctionType
DR = mybir.MatmulPerfMode.DoubleRow

B, N, D = 4, 2048, 512
N_CORES = 8
NQ = N // 2          # 1024 queries per core
M = N                # 2048 keys per core
P = 128              # partitions
SCALE = float(np.sqrt(float(D)))
WSCALE = 64.0        # fp8 weight pre-scale (keeps w~0.02 out of subnormals)

ND = D // P          # 4 contraction chunks of 128
NNT = NQ // P        # 8 query tiles of 128
NMT = M // P         # 16 key tiles of 128
NMC = M // 512       # 4 key chunks of 512
NCH = NQ // 512      # 2 query chunks of 512
NB = NMT // 2        # 8 key blocks of 256 (DoubleRow pairs)


def _sl(i, w=P):
    return slice(i * w, (i + 1) * w)


def _build(with_bqk: bool, with_bv: bool, with_bo: bool):
    nc = bacc.Bacc("TRN2", target_bir_lowering=False, debug=False,
                   num_devices=N_CORES)

    xqT16 = nc.dram_tensor("xqT16", [D, NQ], F16, kind="ExternalInput").ap()
    xkvT16 = nc.dram_tensor("xkvT16", [D, M], F16, kind="ExternalInput").ap()
    xkv8d = nc.dram_tensor("xkv8", [P, ND, M], F8, kind="ExternalInput").ap()
    xq32 = nc.dram_tensor("xq32", [NQ, D], F32, kind="ExternalInput").ap()
    wq = nc.dram_tensor("wq16", [D, D], F16, kind="ExternalInput").ap()
    wk = nc.dram_tensor("wk16", [D, D], F16, kind="ExternalInput").ap()
    wv8d = nc.dram_tensor("wv8", [P, ND, D], F8, kind="ExternalInput").ap()
    wo8d = nc.dram_tensor("wo8", [P, ND, D], F8, kind="ExternalInput").ap()
    bq = nc.dram_tensor("bq", [D], F32, kind="ExternalInput").ap()
    bk = nc.dram_tensor("bk", [D], F32, kind="ExternalInput").ap()
    bv32 = nc.dram_tensor("bv32", [1, D], F32, kind="ExternalInput").ap()
    bo32 = nc.dram_tensor("bo32", [1, D], F32, kind="ExternalInput").ap()
    out = nc.dram_tensor("out", [NQ, D], F32, kind="ExternalOutput").ap()

    with tile.TileContext(nc) as tc:
        with tc.tile_pool(name="pers", bufs=1) as pers:
            # ---- persistent tiles ----------------------------------------------
            WQ = [pers.tile([P, D], F16, name=f"wq{d}", tag=f"wq{d}") for d in range(ND)]
            WK = [pers.tile([P, D], F16, name=f"wk{d}", tag=f"wk{d}") for d in range(ND)]
            WV8 = pers.tile([P, ND, D], F8, name="wv8", tag="wv8")
            WO8 = pers.tile([P, ND, D], F8, name="wo8", tag="wo8")
            XQT = [pers.tile([P, NQ], F16, name=f"xqt{d}", tag=f"xqt{d}") for d in range(ND)]
            XKVT = [pers.tile([P, M], F16, name=f"xkvt{d}", tag=f"xkvt{d}") for d in range(ND)]
            XKV8 = pers.tile([P, ND, M], F8, name="xkv8", tag="xkv8")
            XQ32 = [pers.tile([P, D], F32, name=f"xq32_{t}", tag=f"xq32_{t}") for t in range(NNT)]

            # All loads issue from sync in consumption order. The ~0.6us
            # serial issue cost per dma_start naturally staggers the loads,
            # which keeps DMA descriptors from many tiles from interleaving
            # in the queues (interleaved streams thrash DRAM: measured ~3x
            # slower per descriptor when loads are issued from 3 engines).
            def load(out_ap, in_ap):
                nc.sync.dma_start(out=out_ap, in_=in_ap)

            # issue order = consumption order; big inputs chunked by 512-col
            # slab so the first projection groups release early
            for d in range(ND):
                load(WQ[d][:], wq[_sl(d), :])
                load(XQT[d][:, _sl(0, 512)], xqT16[_sl(d), _sl(0, 512)])
            for d in range(ND):
                load(XQT[d][:, _sl(1, 512)], xqT16[_sl(d), _sl(1, 512)])
            for d in range(ND):
                load(WK[d][:], wk[_sl(d), :])
            for c in range(NMC):
                for d in range(ND):
                    load(XKVT[d][:, _sl(c, 512)], xkvT16[_sl(d), _sl(c, 512)])
            load(XKV8[:], xkv8d[:])
            load(WV8[:], wv8d[:])
            load(WO8[:], wo8d[:])
            for t in range(NNT):
                load(XQ32[t][:], xq32[_sl(t), :])
            if with_bqk:
                BQ = [pers.tile([P, 1], F32, name=f"bq{i}", tag=f"bq{i}") for i in range(ND)]
                BK = [pers.tile([P, 1], F32, name=f"bk{i}", tag=f"bk{i}") for i in range(ND)]
                for i in range(ND):
                    load(BQ[i][:], bq[_sl(i)].rearrange("(a b) -> a b", b=1))
                    load(BK[i][:], bk[_sl(i)].rearrange("(a b) -> a b", b=1))
            if with_bv:
                BV = pers.tile([1, D], F32, name="bv", tag="bv")
                BVB = pers.tile([P, D], F32, name="bvb", tag="bvb")
                INV64 = pers.tile([P, 1], F32, name="inv64", tag="inv64")
                load(BV[:], bv32[:])
                nc.gpsimd.partition_broadcast(BVB[:], BV[:])
                nc.gpsimd.memset(INV64[:], 1.0 / WSCALE)
            if with_bo:
                BO = pers.tile([1, D], F32, name="bo", tag="bo")
                BOB = pers.tile([P, D], F32, name="bob", tag="bob")
                load(BO[:], bo32[:])
                nc.gpsimd.partition_broadcast(BOB[:], BO[:])

            qT = [pers.tile([P, NQ], F16, name=f"qT{a}", tag=f"qT{a}") for a in range(ND)]
            kT = [pers.tile([P, M], F16, name=f"kT{a}", tag=f"kT{a}") for a in range(ND)]
            # V8[m][p, i, c] = v[key 256m + 2p + i, c] in fp8 (parity split,
            # matching the byte layout the u16-packed transpose produces)
            V8 = [pers.tile([P, 2, D], F8, name=f"v8_{m}", tag=f"v8_{m}") for m in range(NB)]
            # P^T in fp8, transposed as packed uint16 pairs: PTBu[p, m, q]
            # holds bytes (P[q, 256m+2p], P[q, 256m+2p+1]).
            PTBu = pers.tile([P, NB, NQ], mybir.dt.uint16, name="PTBu", tag="PTBu")
            PTB8v = PTBu.bitcast(F8)  # [P, NB, 2*NQ], (q, parity) interleaved
            recip = [pers.tile([P, 1], F32, name=f"recip{t}", tag=f"recip{t}") for t in range(NNT)]
            # OT8[p, ct, n] = O^T[128*ct + p, n] in fp8
            OT8 = pers.tile([P, ND, NQ], F8, name="OT8", tag="OT8")

            # PE warm-up scratch: the PE sits idle ~4us waiting for the
            # first input DMAs, and the clock only ramps to full speed after
            # ~3us of continuous execution. Junk matmuls on memset scratch
            # fill the DMA wait so the real Q projection starts at full clock.
            JW = pers.tile([P, P], F16, name="jw", tag="jw")
            JX = pers.tile([P, 512], F16, name="jx", tag="jx")
            nc.gpsimd.memset(JW[:], 0.0)
            nc.gpsimd.memset(JX[:], 0.0)

            # copy engines for projection PSUM->SBUF rotation
            cengs = [nc.scalar.copy, nc.vector.tensor_copy]

            with tc.tile_pool(name="spool", bufs=3, space="PSUM") as spool, \
                 tc.tile_pool(name="otps", bufs=2, space="PSUM") as otps, \
                 tc.tile_pool(name="ppool", bufs=6) as ppool, \
                 tc.tile_pool(name="stat", bufs=8) as stat, \
                 tc.tile_pool(name="fin", bufs=4) as fin:

                jps = spool.tile([P, M // 2], F32, name="jps", tag="S")
                for _ in range(10):
                    nc.tensor.matmul(jps[:, :512], JW[:], JX[:],
                                     start=True, stop=True)

                # ---- Q projection: groups (c, a), accumulate over d ------------
                ci = [0]

                def proj_group(wtiles, xtiles, dst, btiles, c, a, cw):
                    ps = otps.tile([P, 512], F32, name="ot", tag="ot")
                    for d in range(ND):
                        nc.tensor.matmul(ps[:], wtiles[d][:, _sl(a)],
                                         xtiles[d][:, _sl(c, 512)],
                                         start=(d == 0), stop=(d == ND - 1))
                    if with_bqk:
                        nc.vector.tensor_scalar_add(
                            dst[a][:, _sl(c, 512)], ps[:], btiles[a][:])
                    else:
                        cp = cengs[ci[0] % len(cengs)]
                        ci[0] += 1
                        cp(dst[a][:, _sl(c, 512)], ps[:])

                for c in range(NCH):
                    for a in range(ND):
                        proj_group(WQ, XQT, qT, BQ if with_bqk else None, c, a, NCH)
                for c in range(NMC):
                    for a in range(ND):
                        proj_group(WK, XKVT, kT, BK if with_bqk else None, c, a, NMC)

                # ---- V projection: fp8 DoubleRow over paired d-chunks ----------
                # key = 256b + 2k + two: parity-strided view for the V8 split
                XKV8r = XKV8.rearrange("p d (b k two) -> p d b two k",
                                       b=NB, two=2)

                def v_group(m, i):
                    ps = otps.tile([P, 512], F32, name="ot", tag="ot")
                    for j in range(ND // 2):
                        nc.tensor.matmul(
                            ps[:], XKV8r[:, 2 * j:2 * j + 2, m, i, :],
                            WV8[:, 2 * j:2 * j + 2, :],
                            start=(j == 0), stop=(j == ND // 2 - 1),
                            perf_mode=DR)
                    if with_bv:
                        nc.vector.scalar_tensor_tensor(
                            out=V8[m][:, i, :], in0=ps[:], scalar=INV64[:],
                            in1=BVB[:], op0=ALU.mult, op1=ALU.add)
                    elif i == 0:
                        nc.scalar.mul(V8[m][:, i, :], ps[:], 1.0 / WSCALE)
                    else:
                        # split psum->V8 copies across scalar+vector so the
                        # scalar queue enters the softmax phase drained
                        nc.vector.tensor_scalar_mul(V8[m][:, i, :], ps[:],
                                                    1.0 / WSCALE)

                # ---- scores + softmax ------------------------------------------
                def softmax_tile(t):
                    halves = []
                    nmh = []
                    for h in range(2):
                        sps = spool.tile([P, M // 2], F32, name=f"S{h}", tag="S")
                        for a in range(ND):
                            for c in range(2):
                                mc = h * 2 + c
                                nc.tensor.matmul(sps[:, _sl(c, 512)],
                                                 qT[a][:, _sl(t)],
                                                 kT[a][:, _sl(mc, 512)],
                                                 start=(a == 0), stop=(a == ND - 1))
                        nm = stat.tile([P, 1], F32, name=f"negmax{h}", tag=f"negmax{h}")
                        nc.vector.tensor_reduce(nm[:], sps[:], axis=AX.X,
                                                op=ALU.max, negate=True)
                        halves.append(sps)
                        nmh.append(nm)
                    negmax = stat.tile([P, 1], F32, name="negmax", tag="negmax")
                    nc.vector.tensor_tensor(negmax[:], nmh[0][:], nmh[1][:],
                                            op=ALU.min)
                    # exp writes P directly in fp8; the transpose moves packed
                    # uint16 byte-pairs (fp8 isn't a legal xbar dtype), which
                    # lands adjacent keys in one partition's byte lanes --
                    # exactly the DoubleRow rhs layout PV wants.
                    pt8 = ppool.tile([P, M], F8, name="P", tag="P")
                    rsh = []
                    for h in range(2):
                        # exp -> half-transpose issued immediately so the last
                        # tiles' P^T chain is short (it gates PV).
                        rs = stat.tile([P, 1], F32, name=f"rowsum{h}", tag=f"rowsum{h}")
                        nc.scalar.activation(pt8[:, _sl(h, M // 2)], halves[h][:],
                                             ACTF.Exp, bias=negmax[:], scale=1.0,
                                             accum_out=rs[:])
                        rsh.append(rs)
                        hb = slice(h * (NB // 2), (h + 1) * (NB // 2))
                        nc.sync.dma_start(
                            out=PTBu[:, hb, _sl(t)],
                            in_=pt8[:, _sl(h, M // 2)].bitcast(mybir.dt.uint16),
                            transpose=True)
                    rowsum = stat.tile([P, 1], F32, name="rowsum", tag="rowsum")
                    nc.vector.tensor_tensor(rowsum[:], rsh[0][:], rsh[1][:],
                                            op=ALU.add)
                    rs64 = stat.tile([P, 1], F32, name="rs64", tag="rs64")
                    nc.vector.tensor_scalar_mul(rs64[:], rowsum[:], WSCALE)
                    nc.vector.reciprocal(recip[t][:], rs64[:])

                # V-projection groups interleave between the early score
                # tiles: the PE chews V matmuls while each tile's max->exp
                # chain releases its score psum ring slots (otherwise the
                # ring-3 release latency stalls the PE ~1us per tile).
                for t in range(NNT):
                    softmax_tile(t)
                    if t < 4:
                        for mt in range(4 * t, 4 * t + 4):
                            v_group(mt // 2, mt % 2)

                # ---- PV (fp8 DoubleRow over paired key tiles) ------------------
                def pv_chunk(ck):
                    # ck0's copies ride vector (scalar is still draining the
                    # softmax-tail exps), ck1's ride the then-free scalar.
                    for ct in range(ND):
                        ps = otps.tile([P, 512], F32, name="ot", tag="ot")
                        for m in range(NB):
                            rhs = PTB8v[:, m, :].rearrange(
                                "p (q b) -> p b q", b=2)[:, :, _sl(ck, 512)]
                            nc.tensor.matmul(
                                ps[:], V8[m][:, :, _sl(ct)], rhs,
                                start=(m == 0), stop=(m == NB - 1),
                                perf_mode=DR)
                        nc.scalar.copy(OT8[:, ct, _sl(ck, 512)], ps[:])

                # ---- output projection (fp8 DoubleRow) -------------------------
                def y_tile(t):
                    ps = otps.tile([P, D], F32, name="y", tag="ot")
                    for j in range(ND // 2):
                        nc.tensor.matmul(
                            ps[:], OT8[:, 2 * j:2 * j + 2, _sl(t)],
                            WO8[:, 2 * j:2 * j + 2, :],
                            start=(j == 0), stop=(j == ND // 2 - 1),
                            perf_mode=DR)
                    osb = fin.tile([P, D], F32, name="osb", tag="osb")
                    nc.vector.scalar_tensor_tensor(
                        out=osb[:], in0=ps[:], scalar=recip[t][:],
                        in1=XQ32[t][:], op0=ALU.mult, op1=ALU.add)
                    if with_bo:
                        nc.vector.tensor_add(osb[:], osb[:], BOB[:])
                    nc.sync.dma_start(out=out[_sl(t), :], in_=osb[:])

                pv_chunk(0)
                for t in range(NNT // 2):
                    y_tile(t)
                # The PE idles ~5us here waiting for the last score tile's
                # transposes (parked-queue semaphore wake latency); junk
                # matmuls keep the clock at full speed so PV1 doesn't pay
                # the downclocked rate (~267 vs 221 ns/matmul) afterwards.
                jps2 = spool.tile([P, M // 2], F32, name="jps2", tag="S")
                for _ in range(12):
                    nc.tensor.matmul(jps2[:, :512], JW[:], JX[:],
                                     start=True, stop=True)
                pv_chunk(1)
                for t in range(NNT // 2, NNT):
                    y_tile(t)

    nc.compile()
    return nc


_BUILD_CACHE = {}


def _get_nc(with_bqk: bool, with_bv: bool, with_bo: bool):
    key = (with_bqk, with_bv, with_bo)
    if key not in _BUILD_CACHE:
        _BUILD_CACHE[key] = _build(with_bqk, with_bv, with_bo)
    return _BUILD_CACHE[key]


def kernel(query, key_value, Wq, bq, Wk, bk, Wv, bv, Wo, bo, _timing=None):
    query = np.asarray(query, dtype=np.float32)
    key_value = np.asarray(key_value, dtype=np.float32)
    Wq = np.asarray(Wq, dtype=np.float32)
    Wk = np.asarray(Wk, dtype=np.float32)
    Wv = np.asarray(Wv, dtype=np.float32)
    Wo = np.asarray(Wo, dtype=np.float32)
    bq = np.asarray(bq, dtype=np.float32)
    bk = np.asarray(bk, dtype=np.float32)
    bv = np.asarray(bv, dtype=np.float32)
    bo = np.asarray(bo, dtype=np.float32)

    with_bqk = bool(np.any(bq)) or bool(np.any(bk))
    with_bv = bool(np.any(bv))
    with_bo = bool(np.any(bo))
    nc = _get_nc(with_bqk, with_bv, with_bo)

    f8 = ml_dtypes.float8_e4m3fn
    wq16 = (Wq * SCALE).astype(np.float16)
    wk16 = Wk.astype(np.float16)
    # [128, 4, 512] chunked layouts for fp8 weights, pre-scaled by 64
    wv8 = np.ascontiguousarray(
        (Wv * WSCALE).astype(f8).reshape(ND, P, D).transpose(1, 0, 2))
    wo8 = np.ascontiguousarray(
        (Wo * WSCALE).astype(f8).reshape(ND, P, D).transpose(1, 0, 2))
    bqs = (bq * SCALE).astype(np.float32)
    bk32 = bk.astype(np.float32)
    bv32 = bv.astype(np.float32).reshape(1, D)
    bo32 = bo.astype(np.float32).reshape(1, D)

    q16 = query.astype(np.float16)
    kv16 = key_value.astype(np.float16)
    kv8 = key_value.astype(f8)

    in_maps = []
    for core in range(N_CORES):
        b, h = divmod(core, 2)
        sl = slice(h * NQ, (h + 1) * NQ)
        im = {
            "xqT16": np.ascontiguousarray(q16[b, sl].T),
            "xkvT16": np.ascontiguousarray(kv16[b].T),
            # [128, 4, 2048]: xkv8[p, j, key] = kv[key, 128j+p]
            "xkv8": np.ascontiguousarray(
                kv8[b].T.reshape(ND, P, M).transpose(1, 0, 2)),
            "xq32": np.ascontiguousarray(query[b, sl]),
            "wq16": wq16, "wk16": wk16, "wv8": wv8, "wo8": wo8,
            "bq": bqs, "bk": bk32, "bv32": bv32, "bo32": bo32,
        }
        in_maps.append(im)

    res = run_bass_kernel_spmd(nc, in_maps, list(range(N_CORES)),
                               **(_timing or {}))
    out = np.empty((B, N, D), dtype=np.float32)
    for core in range(N_CORES):
        b, h = divmod(core, 2)
        out[b, h * NQ:(h + 1) * NQ] = res.results[core]["out"]
    if _timing is not None:
        return out, res
    return out


# revision 52
# speedup vs baseline: 1.0088x; 1.0088x over previous
"""AxisAttention TRN2 Bass kernel.

Full-input contract: kernel(**inputs) takes the unsharded numpy inputs and
returns the full [4, 2048, 512] float32 output.

Sharding: data-parallel over (batch, query-half) -> 8 NeuronCores. Each core
computes attention for 1024 queries of one batch against that batch's full
2048 keys. Params are replicated. K/V projections are recomputed by the two
cores sharing a batch (cheaper than a cross-core exchange).

Math per core (n=1024 queries, m=2048 keys, d=a=c=512):
  qT[a,n]  = sum_d WqS[d,a] * xqT[d,n]          (WqS = Wq*sqrt(512), fp16)
  kT[a,m]  = sum_d Wk[d,a] * xkvT[d,m]          (fp16)
  v[m,c]   = sum_d xkv8[d,m] * Wv8[d,c] / 64    (fp8 DoubleRow, Wv8=Wv*64)
  S[n,m]   = sum_a qT[a,n] * kT[a,m]            (fp16, PSUM f32)
  P[n,m]   = exp(S - rowmax(S)) in fp8; rowsum via ACT accum (f32)
  PT[m,n]  = DMA-xbar transpose of P as packed uint16 key-pairs
  OT[c,n]  = sum_m v[m,c] * PT[m,n]             (fp8 DoubleRow)
  YT[dq,n] = sum_c Wo8[c,dq] * OT[c,n]          (fp8 DoubleRow, Wo8=Wo*64)
  out[n,:] = Y * (1/(64*rowsum))[n] + query32[n,:]  (+bo broadcast)

Precision split: the score path (q/k projections, S) stays fp16 because the
reference multiplies scores by sqrt(512) -> logits with std ~100, so softmax
is near-argmax and tiny score errors flip rows. The value path (v, P*v, out
projection) tolerates fp8e4m3; weights are pre-scaled by 64 so w~0.02 values
stay out of fp8 subnormal range, descaled in the f32 epilogue.

Scheduling: one PSUM pool region for everything (pool transitions insert
multi-us barriers). Tag "S" = 3 x [128,1024] score buffers (6 banks); tag
"ot" = 2 x [128,512] (2 banks) shared by all projection / PV / Y psums.
All loads issue from the sync queue in consumption order (staggered issue
keeps DMA descriptor streams from thrashing DRAM); exp -> half-transpose
is issued per score half so the last tiles' P^T chain stays short. The
P^T transpose moves packed uint16 byte-pairs of the fp8 P, which lands
adjacent keys in one partition's byte lanes -- exactly the interleaved
rhs layout the DoubleRow PV matmul consumes, so no separate cast pass
exists. PSUM->SBUF copies are split across the scalar and vector queues
so neither becomes a convoy for the softmax-tail exps that gate PV.
"""

import numpy as np
import ml_dtypes

import concourse.bass as bass
import concourse.mybir as mybir
import concourse.tile as tile
from concourse import bacc
from concourse.bass_utils import run_bass_kernel_spmd

F8 = mybir.dt.float8e4
F16 = mybir.dt.float16
F32 = mybir.dt.float32
AX = mybir.AxisListType
ALU = mybir.AluOpType
ACTF = mybir.ActivationFunctionType
DR = mybir.MatmulPerfMode.DoubleRow

B, N, D = 4, 2048, 512
N_CORES = 8
NQ = N // 2          # 1024 queries per core
M = N                # 2048 keys per core
P = 128              # partitions
SCALE = float(np.sqrt(float(D)))
WSCALE = 64.0        # fp8 weight pre-scale (keeps w~0.02 out of subnormals)

ND = D // P          # 4 contraction chunks of 128
NNT = NQ // P        # 8 query tiles of 128
NMT = M // P         # 16 key tiles of 128
NMC = M // 512       # 4 key chunks of 512
NCH = NQ // 512      # 2 query chunks of 512
NB = NMT // 2        # 8 key blocks of 256 (DoubleRow pairs)


def _sl(i, w=P):
    return slice(i * w, (i + 1) * w)


def _build(with_bqk: bool, with_bv: bool, with_bo: bool):
    nc = bacc.Bacc("TRN2", target_bir_lowering=False, debug=False,
                   num_devices=N_CORES)

    xqT16 = nc.dram_tensor("xqT16", [D, NQ], F16, kind="ExternalInput").ap()
    xkvT16 = nc.dram_tensor("xkvT16", [D, M], F16, kind="ExternalInput").ap()
    xkv8d = nc.dram_tensor("xkv8", [P, ND, M], F8, kind="ExternalInput").ap()
    xq32 = nc.dram_tensor("xq32", [NQ, D], F32, kind="ExternalInput").ap()
    wq = nc.dram_tensor("wq16", [D, D], F16, kind="ExternalInput").ap()
    wk = nc.dram_tensor("wk16", [D, D], F16, kind="ExternalInput").ap()
    wv8d = nc.dram_tensor("wv8", [P, ND, D], F8, kind="ExternalInput").ap()
    wo8d = nc.dram_tensor("wo8", [P, ND, D], F8, kind="ExternalInput").ap()
    bq = nc.dram_tensor("bq", [D], F32, kind="ExternalInput").ap()
    bk = nc.dram_tensor("bk", [D], F32, kind="ExternalInput").ap()
    bv32 = nc.dram_tensor("bv32", [1, D], F32, kind="ExternalInput").ap()
    bo32 = nc.dram_tensor("bo32", [1, D], F32, kind="ExternalInput").ap()
    out = nc.dram_tensor("out", [NQ, D], F32, kind="ExternalOutput").ap()

    with tile.TileContext(nc) as tc:
        with tc.tile_pool(name="pers", bufs=1) as pers:
            # ---- persistent tiles ----------------------------------------------
            WQ = [pers.tile([P, D], F16, name=f"wq{d}", tag=f"wq{d}") for d in range(ND)]
            WK = [pers.tile([P, D], F16, name=f"wk{d}", tag=f"wk{d}") for d in range(ND)]
            WV8 = pers.tile([P, ND, D], F8, name="wv8", tag="wv8")
            WO8 = pers.tile([P, ND, D], F8, name="wo8", tag="wo8")
            XQT = [pers.tile([P, NQ], F16, name=f"xqt{d}", tag=f"xqt{d}") for d in range(ND)]
            XKVT = [pers.tile([P, M], F16, name=f"xkvt{d}", tag=f"xkvt{d}") for d in range(ND)]
            XKV8 = pers.tile([P, ND, M], F8, name="xkv8", tag="xkv8")
            XQ32 = [pers.tile([P, D], F32, name=f"xq32_{t}", tag=f"xq32_{t}") for t in range(NNT)]

            # All loads issue from sync in consumption order. The ~0.6us
            # serial issue cost per dma_start naturally staggers the loads,
            # which keeps DMA descriptors from many tiles from interleaving
            # in the queues (interleaved streams thrash DRAM: measured ~3x
            # slower per descriptor when loads are issued from 3 engines).
            def load(out_ap, in_ap):
                nc.sync.dma_start(out=out_ap, in_=in_ap)

            # issue order = consumption order; big inputs chunked by 512-col
            # slab so the first projection groups release early
            for d in range(ND):
                load(WQ[d][:], wq[_sl(d), :])
                load(XQT[d][:, _sl(0, 512)], xqT16[_sl(d), _sl(0, 512)])
            for d in range(ND):
                load(XQT[d][:, _sl(1, 512)], xqT16[_sl(d), _sl(1, 512)])
            for d in range(ND):
                load(WK[d][:], wk[_sl(d), :])
            for c in range(NMC):
                for d in range(ND):
                    load(XKVT[d][:, _sl(c, 512)], xkvT16[_sl(d), _sl(c, 512)])
            load(XKV8[:], xkv8d[:])
            load(WV8[:], wv8d[:])
            load(WO8[:], wo8d[:])
            for t in range(NNT):
                load(XQ32[t][:], xq32[_sl(t), :])
            if with_bqk:
                BQ = [pers.tile([P, 1], F32, name=f"bq{i}", tag=f"bq{i}") for i in range(ND)]
                BK = [pers.tile([P, 1], F32, name=f"bk{i}", tag=f"bk{i}") for i in range(ND)]
                for i in range(ND):
                    load(BQ[i][:], bq[_sl(i)].rearrange("(a b) -> a b", b=1))
                    load(BK[i][:], bk[_sl(i)].rearrange("(a b) -> a b", b=1))
            if with_bv:
                BV = pers.tile([1, D], F32, name="bv", tag="bv")
                BVB = pers.tile([P, D], F32, name="bvb", tag="bvb")
                INV64 = pers.tile([P, 1], F32, name="inv64", tag="inv64")
                load(BV[:], bv32[:])
                nc.gpsimd.partition_broadcast(BVB[:], BV[:])
                nc.gpsimd.memset(INV64[:], 1.0 / WSCALE)
            if with_bo:
                BO = pers.tile([1, D], F32, name="bo", tag="bo")
                BOB = pers.tile([P, D], F32, name="bob", tag="bob")
                load(BO[:], bo32[:])
                nc.gpsimd.partition_broadcast(BOB[:], BO[:])

            qT = [pers.tile([P, NQ], F16, name=f"qT{a}", tag=f"qT{a}") for a in range(ND)]
            kT = [pers.tile([P, M], F16, name=f"kT{a}", tag=f"kT{a}") for a in range(ND)]
            # V8[m][p, i, c] = v[key 256m + 2p + i, c] in fp8 (parity split,
            # matching the byte layout the u16-packed transpose produces)
            V8 = [pers.tile([P, 2, D], F8, name=f"v8_{m}", tag=f"v8_{m}") for m in range(NB)]
            # P^T in fp8, transposed as packed uint16 pairs: PTBu[p, m, q]
            # holds bytes (P[q, 256m+2p], P[q, 256m+2p+1]).
            PTBu = pers.tile([P, NB, NQ], mybir.dt.uint16, name="PTBu", tag="PTBu")
            PTB8v = PTBu.bitcast(F8)  # [P, NB, 2*NQ], (q, parity) interleaved
            recip = [pers.tile([P, 1], F32, name=f"recip{t}", tag=f"recip{t}") for t in range(NNT)]
            # OT8[p, ct, n] = O^T[128*ct + p, n] in fp8
            OT8 = pers.tile([P, ND, NQ], F8, name="OT8", tag="OT8")

            # PE warm-up scratch: the PE sits idle ~4us waiting for the
            # first input DMAs, and the clock only ramps to full speed after
            # ~3us of continuous execution. Junk matmuls on memset scratch
            # fill the DMA wait so the real Q projection starts at full clock.
            JW = pers.tile([P, P], F16, name="jw", tag="jw")
            JX = pers.tile([P, 512], F16, name="jx", tag="jx")
            nc.gpsimd.memset(JW[:], 0.0)
            nc.gpsimd.memset(JX[:], 0.0)

            # copy engines for projection PSUM->SBUF rotation
            cengs = [nc.scalar.copy, nc.vector.tensor_copy]

            with tc.tile_pool(name="spool", bufs=3, space="PSUM") as spool, \
                 tc.tile_pool(name="otps", bufs=2, space="PSUM") as otps, \
                 tc.tile_pool(name="ppool", bufs=6) as ppool, \
                 tc.tile_pool(name="stat", bufs=8) as stat, \
                 tc.tile_pool(name="fin", bufs=4) as fin:

                jps = spool.tile([P, M // 2], F32, name="jps", tag="S")
                for _ in range(10):
                    nc.tensor.matmul(jps[:, :512], JW[:], JX[:],
                                     start=True, stop=True)

                # ---- Q projection: groups (c, a), accumulate over d ------------
                ci = [0]

                def proj_group(wtiles, xtiles, dst, btiles, c, a, cw):
                    ps = otps.tile([P, 512], F32, name="ot", tag="ot")
                    for d in range(ND):
                        nc.tensor.matmul(ps[:], wtiles[d][:, _sl(a)],
                                         xtiles[d][:, _sl(c, 512)],
                                         start=(d == 0), stop=(d == ND - 1))
                    if with_bqk:
                        nc.vector.tensor_scalar_add(
                            dst[a][:, _sl(c, 512)], ps[:], btiles[a][:])
                    else:
                        cp = cengs[ci[0] % len(cengs)]
                        ci[0] += 1
                        cp(dst[a][:, _sl(c, 512)], ps[:])

                for c in range(NCH):
                    for a in range(ND):
                        proj_group(WQ, XQT, qT, BQ if with_bqk else None, c, a, NCH)
                for c in range(NMC):
                    for a in range(ND):
                        proj_group(WK, XKVT, kT, BK if with_bqk else None, c, a, NMC)

                # ---- V projection: fp8 DoubleRow over paired d-chunks ----------
                # key = 256b + 2k + two: parity-strided view for the V8 split
                XKV8r = XKV8.rearrange("p d (b k two) -> p d b two k",
                                       b=NB, two=2)

                def v_group(m, i):
                    ps = otps.tile([P, 512], F32, name="ot", tag="ot")
                    for j in range(ND // 2):
                        nc.tensor.matmul(
                            ps[:], XKV8r[:, 2 * j:2 * j + 2, m, i, :],
                            WV8[:, 2 * j:2 * j + 2, :],
                            start=(j == 0), stop=(j == ND // 2 - 1),
                            perf_mode=DR)
                    if with_bv:
                        nc.vector.scalar_tensor_tensor(
                            out=V8[m][:, i, :], in0=ps[:], scalar=INV64[:],
                            in1=BVB[:], op0=ALU.mult, op1=ALU.add)
                    elif i == 0:
                        nc.scalar.mul(V8[m][:, i, :], ps[:], 1.0 / WSCALE)
                    else:
                        # split psum->V8 copies across scalar+vector so the
                        # scalar queue enters the softmax phase drained
                        nc.vector.tensor_scalar_mul(V8[m][:, i, :], ps[:],
                                                    1.0 / WSCALE)

                # ---- scores + softmax ------------------------------------------
                def softmax_tile(t):
                    halves = []
                    nmh = []
                    for h in range(2):
                        sps = spool.tile([P, M // 2], F32, name=f"S{h}", tag="S")
                        for a in range(ND):
                            for c in range(2):
                                mc = h * 2 + c
                                nc.tensor.matmul(sps[:, _sl(c, 512)],
                                                 qT[a][:, _sl(t)],
                                                 kT[a][:, _sl(mc, 512)],
                                                 start=(a == 0), stop=(a == ND - 1))
                        nm = stat.tile([P, 1], F32, name=f"negmax{h}", tag=f"negmax{h}")
                        nc.vector.tensor_reduce(nm[:], sps[:], axis=AX.X,
                                                op=ALU.max, negate=True)
                        halves.append(sps)
                        nmh.append(nm)
                    negmax = stat.tile([P, 1], F32, name="negmax", tag="negmax")
                    nc.vector.tensor_tensor(negmax[:], nmh[0][:], nmh[1][:],
                                            op=ALU.min)
                    # exp writes P directly in fp8; the transpose moves packed
                    # uint16 byte-pairs (fp8 isn't a legal xbar dtype), which
                    # lands adjacent keys in one partition's byte lanes --
                    # exactly the DoubleRow rhs layout PV wants.
                    pt8 = ppool.tile([P, M], F8, name="P", tag="P")
                    rsh = []
                    for h in range(2):
                        # exp -> half-transpose issued immediately so the last
                        # tiles' P^T chain is short (it gates PV).
                        rs = stat.tile([P, 1], F32, name=f"rowsum{h}", tag=f"rowsum{h}")
                        nc.scalar.activation(pt8[:, _sl(h, M // 2)], halves[h][:],
                                             ACTF.Exp, bias=negmax[:], scale=1.0,
                                             accum_out=rs[:])
                        rsh.append(rs)
                        hb = slice(h * (NB // 2), (h + 1) * (NB // 2))
                        nc.sync.dma_start(
                            out=PTBu[:, hb, _sl(t)],
                            in_=pt8[:, _sl(h, M // 2)].bitcast(mybir.dt.uint16),
                            transpose=True)
                    rowsum = stat.tile([P, 1], F32, name="rowsum", tag="rowsum")
                    nc.vector.tensor_tensor(rowsum[:], rsh[0][:], rsh[1][:],
                                            op=ALU.add)
                    rs64 = stat.tile([P, 1], F32, name="rs64", tag="rs64")
                    nc.vector.tensor_scalar_mul(rs64[:], rowsum[:], WSCALE)
                    nc.vector.reciprocal(recip[t][:], rs64[:])

                # V-projection groups interleave between the early score
                # tiles: the PE chews V matmuls while each tile's max->exp
                # chain releases its score psum ring slots (otherwise the
                # ring-3 release latency stalls the PE ~1us per tile).
                for t in range(NNT):
                    softmax_tile(t)
                    if t < 4:
                        for mt in range(4 * t, 4 * t + 4):
                            v_group(mt // 2, mt % 2)

                # ---- PV (fp8 DoubleRow over paired key tiles) ------------------
                def pv_chunk(ck):
                    # ck0's copies ride vector (scalar is still draining the
                    # softmax-tail exps), ck1's ride the then-free scalar.
                    for ct in range(ND):
                        ps = otps.tile([P, 512], F32, name="ot", tag="ot")
                        for m in range(NB):
                            rhs = PTB8v[:, m, :].rearrange(
                                "p (q b) -> p b q", b=2)[:, :, _sl(ck, 512)]
                            nc.tensor.matmul(
                                ps[:], V8[m][:, :, _sl(ct)], rhs,
                                start=(m == 0), stop=(m == NB - 1),
                                perf_mode=DR)
                        nc.scalar.copy(OT8[:, ct, _sl(ck, 512)], ps[:])

                # ---- output projection (fp8 DoubleRow) -------------------------
                def y_tile(t):
                    ps = otps.tile([P, D], F32, name="y", tag="ot")
                    for j in range(ND // 2):
                        nc.tensor.matmul(
                            ps[:], OT8[:, 2 * j:2 * j + 2, _sl(t)],
                            WO8[:, 2 * j:2 * j + 2, :],
                            start=(j == 0), stop=(j == ND // 2 - 1),
                            perf_mode=DR)
                    osb = fin.tile([P, D], F32, name="osb", tag="osb")
                    nc.vector.scalar_tensor_tensor(
                        out=osb[:], in0=ps[:], scalar=recip[t][:],
                        in1=XQ32[t][:], op0=ALU.mult, op1=ALU.add)
                    if with_bo:
                        nc.vector.tensor_add(osb[:], osb[:], BOB[:])
                    nc.sync.dma_start(out=out[_sl(t), :], in_=osb[:])

                pv_chunk(0)
                for t in range(NNT // 2):
                    y_tile(t)
                pv_chunk(1)
                for t in range(NNT // 2, NNT):
                    y_tile(t)

    nc.compile()
    return nc


_BUILD_CACHE = {}


def _get_nc(with_bqk: bool, with_bv: bool, with_bo: bool):
    key = (with_bqk, with_bv, with_bo)
    if key not in _BUILD_CACHE:
        _BUILD_CACHE[key] = _build(with_bqk, with_bv, with_bo)
    return _BUILD_CACHE[key]


def kernel(query, key_value, Wq, bq, Wk, bk, Wv, bv, Wo, bo, _timing=None):
    query = np.asarray(query, dtype=np.float32)
    key_value = np.asarray(key_value, dtype=np.float32)
    Wq = np.asarray(Wq, dtype=np.float32)
    Wk = np.asarray(Wk, dtype=np.float32)
    Wv = np.asarray(Wv, dtype=np.float32)
    Wo = np.asarray(Wo, dtype=np.float32)
    bq = np.asarray(bq, dtype=np.float32)
    bk = np.asarray(bk, dtype=np.float32)
    bv = np.asarray(bv, dtype=np.float32)
    bo = np.asarray(bo, dtype=np.float32)

    with_bqk = bool(np.any(bq)) or bool(np.any(bk))
    with_bv = bool(np.any(bv))
    with_bo = bool(np.any(bo))
    nc = _get_nc(with_bqk, with_bv, with_bo)

    f8 = ml_dtypes.float8_e4m3fn
    wq16 = (Wq * SCALE).astype(np.float16)
    wk16 = Wk.astype(np.float16)
    # [128, 4, 512] chunked layouts for fp8 weights, pre-scaled by 64
    wv8 = np.ascontiguousarray(
        (Wv * WSCALE).astype(f8).reshape(ND, P, D).transpose(1, 0, 2))
    wo8 = np.ascontiguousarray(
        (Wo * WSCALE).astype(f8).reshape(ND, P, D).transpose(1, 0, 2))
    bqs = (bq * SCALE).astype(np.float32)
    bk32 = bk.astype(np.float32)
    bv32 = bv.astype(np.float32).reshape(1, D)
    bo32 = bo.astype(np.float32).reshape(1, D)

    q16 = query.astype(np.float16)
    kv16 = key_value.astype(np.float16)
    kv8 = key_value.astype(f8)

    in_maps = []
    for core in range(N_CORES):
        b, h = divmod(core, 2)
        sl = slice(h * NQ, (h + 1) * NQ)
        im = {
            "xqT16": np.ascontiguousarray(q16[b, sl].T),
            "xkvT16": np.ascontiguousarray(kv16[b].T),
            # [128, 4, 2048]: xkv8[p, j, key] = kv[key, 128j+p]
            "xkv8": np.ascontiguousarray(
                kv8[b].T.reshape(ND, P, M).transpose(1, 0, 2)),
            "xq32": np.ascontiguousarray(query[b, sl]),
            "wq16": wq16, "wk16": wk16, "wv8": wv8, "wo8": wo8,
            "bq": bqs, "bk": bk32, "bv32": bv32, "bo32": bo32,
        }
        in_maps.append(im)

    res = run_bass_kernel_spmd(nc, in_maps, list(range(N_CORES)),
                               **(_timing or {}))
    out = np.empty((B, N, D), dtype=np.float32)
    for core in range(N_CORES):
        b, h = divmod(core, 2)
        out[b, h * NQ:(h + 1) * NQ] = res.results[core]["out"]
    if _timing is not None:
        return out, res
    return out


# revision 53
# speedup vs baseline: 1.1828x; 1.1725x over previous
"""AxisAttention TRN2 Bass kernel.

Full-input contract: kernel(**inputs) takes the unsharded numpy inputs and
returns the full [4, 2048, 512] float32 output.

Sharding: data-parallel over (batch, query-half) -> 8 NeuronCores. Each core
computes attention for 1024 queries of one batch against that batch's full
2048 keys. Params are replicated. K/V projections are recomputed by the two
cores sharing a batch (cheaper than a cross-core exchange).

Math per core (n=1024 queries, m=2048 keys, d=a=c=512):
  qT[a,n]  = sum_d WqS[d,a] * xqT[d,n]          (WqS = Wq*sqrt(512), fp16)
  kT[a,m]  = sum_d Wk[d,a] * xkvT[d,m]          (fp16)
  v[m,c]   = sum_d xkv8[d,m] * Wv8[d,c] / 64    (fp8 DoubleRow, Wv8=Wv*64)
  S[n,m]   = sum_a qT[a,n] * kT[a,m]            (fp16, PSUM f32)
  P[n,m]   = exp(S - rowmax(S)) in fp8; rowsum via ACT accum (f32)
  PT[m,n]  = DMA-xbar transpose of P as packed uint16 key-pairs
  OT[c,n]  = sum_m v[m,c] * PT[m,n]             (fp8 DoubleRow)
  YT[dq,n] = sum_c Wo8[c,dq] * OT[c,n]          (fp8 DoubleRow, Wo8=Wo*64)
  out[n,:] = Y * (1/(64*rowsum))[n] + query32[n,:]  (+bo broadcast)

Precision split: the score path (q/k projections, S) stays fp16 because the
reference multiplies scores by sqrt(512) -> logits with std ~100, so softmax
is near-argmax and tiny score errors flip rows. The value path (v, P*v, out
projection) tolerates fp8e4m3; weights are pre-scaled by 64 so w~0.02 values
stay out of fp8 subnormal range, descaled in the f32 epilogue.

Scheduling: one PSUM pool region for everything (pool transitions insert
multi-us barriers). Tag "S" = 3 x [128,1024] score buffers (6 banks); tag
"ot" = 2 x [128,512] (2 banks) shared by all projection / PV / Y psums.
All loads issue from the sync queue in consumption order (staggered issue
keeps DMA descriptor streams from thrashing DRAM); exp -> half-transpose
is issued per score half so the last tiles' P^T chain stays short. The
P^T transpose moves packed uint16 byte-pairs of the fp8 P, which lands
adjacent keys in one partition's byte lanes -- exactly the interleaved
rhs layout the DoubleRow PV matmul consumes, so no separate cast pass
exists. PSUM->SBUF copies are split across the scalar and vector queues
so neither becomes a convoy for the softmax-tail exps that gate PV.
"""

import numpy as np
import ml_dtypes

import concourse.bass as bass
import concourse.mybir as mybir
import concourse.tile as tile
from concourse import bacc
from concourse.bass_utils import run_bass_kernel_spmd

F8 = mybir.dt.float8e4
F16 = mybir.dt.float16
F32 = mybir.dt.float32
AX = mybir.AxisListType
ALU = mybir.AluOpType
ACTF = mybir.ActivationFunctionType
DR = mybir.MatmulPerfMode.DoubleRow

B, N, D = 4, 2048, 512
N_CORES = 8
NQ = N // 2          # 1024 queries per core
M = N                # 2048 keys per core
P = 128              # partitions
SCALE = float(np.sqrt(float(D)))
WSCALE = 64.0        # fp8 weight pre-scale (keeps w~0.02 out of subnormals)

ND = D // P          # 4 contraction chunks of 128
NNT = NQ // P        # 8 query tiles of 128
NMT = M // P         # 16 key tiles of 128
NMC = M // 512       # 4 key chunks of 512
NCH = NQ // 512      # 2 query chunks of 512
NB = NMT // 2        # 8 key blocks of 256 (DoubleRow pairs)


def _sl(i, w=P):
    return slice(i * w, (i + 1) * w)


def _build(with_bqk: bool, with_bv: bool, with_bo: bool):
    nc = bacc.Bacc("TRN2", target_bir_lowering=False, debug=False,
                   num_devices=N_CORES)

    xqT16 = nc.dram_tensor("xqT16", [D, NQ], F16, kind="ExternalInput").ap()
    xkvT16 = nc.dram_tensor("xkvT16", [D, M], F16, kind="ExternalInput").ap()
    xkv8d = nc.dram_tensor("xkv8", [P, ND, M], F8, kind="ExternalInput").ap()
    xq32 = nc.dram_tensor("xq32", [NQ, D], F32, kind="ExternalInput").ap()
    wq = nc.dram_tensor("wq16", [D, D], F16, kind="ExternalInput").ap()
    wk = nc.dram_tensor("wk16", [D, D], F16, kind="ExternalInput").ap()
    wv8d = nc.dram_tensor("wv8", [P, ND, D], F8, kind="ExternalInput").ap()
    wo8d = nc.dram_tensor("wo8", [P, ND, D], F8, kind="ExternalInput").ap()
    bq = nc.dram_tensor("bq", [D], F32, kind="ExternalInput").ap()
    bk = nc.dram_tensor("bk", [D], F32, kind="ExternalInput").ap()
    bv32 = nc.dram_tensor("bv32", [1, D], F32, kind="ExternalInput").ap()
    bo32 = nc.dram_tensor("bo32", [1, D], F32, kind="ExternalInput").ap()
    out = nc.dram_tensor("out", [NQ, D], F32, kind="ExternalOutput").ap()

    with tile.TileContext(nc) as tc:
        with tc.tile_pool(name="pers", bufs=1) as pers:
            # ---- persistent tiles ----------------------------------------------
            WQ = [pers.tile([P, D], F16, name=f"wq{d}", tag=f"wq{d}") for d in range(ND)]
            WK = [pers.tile([P, D], F16, name=f"wk{d}", tag=f"wk{d}") for d in range(ND)]
            WV8 = pers.tile([P, ND, D], F8, name="wv8", tag="wv8")
            WO8 = pers.tile([P, ND, D], F8, name="wo8", tag="wo8")
            XQT = [pers.tile([P, NQ], F16, name=f"xqt{d}", tag=f"xqt{d}") for d in range(ND)]
            XKVT = [pers.tile([P, M], F16, name=f"xkvt{d}", tag=f"xkvt{d}") for d in range(ND)]
            XKV8 = pers.tile([P, ND, M], F8, name="xkv8", tag="xkv8")
            XQ32 = [pers.tile([P, D], F32, name=f"xq32_{t}", tag=f"xq32_{t}") for t in range(NNT)]

            # All loads issue from sync in consumption order. The ~0.6us
            # serial issue cost per dma_start naturally staggers the loads,
            # which keeps DMA descriptors from many tiles from interleaving
            # in the queues (interleaved streams thrash DRAM: measured ~3x
            # slower per descriptor when loads are issued from 3 engines).
            def load(out_ap, in_ap):
                nc.sync.dma_start(out=out_ap, in_=in_ap)

            # issue order = consumption order; big inputs chunked by 512-col
            # slab so the first projection groups release early
            for d in range(ND):
                load(WQ[d][:], wq[_sl(d), :])
                load(XQT[d][:, _sl(0, 512)], xqT16[_sl(d), _sl(0, 512)])
            for d in range(ND):
                load(XQT[d][:, _sl(1, 512)], xqT16[_sl(d), _sl(1, 512)])
            for d in range(ND):
                load(WK[d][:], wk[_sl(d), :])
            for c in range(NMC):
                for d in range(ND):
                    load(XKVT[d][:, _sl(c, 512)], xkvT16[_sl(d), _sl(c, 512)])
            load(XKV8[:], xkv8d[:])
            load(WV8[:], wv8d[:])
            load(WO8[:], wo8d[:])
            for t in range(NNT):
                load(XQ32[t][:], xq32[_sl(t), :])
            if with_bqk:
                BQ = [pers.tile([P, 1], F32, name=f"bq{i}", tag=f"bq{i}") for i in range(ND)]
                BK = [pers.tile([P, 1], F32, name=f"bk{i}", tag=f"bk{i}") for i in range(ND)]
                for i in range(ND):
                    load(BQ[i][:], bq[_sl(i)].rearrange("(a b) -> a b", b=1))
                    load(BK[i][:], bk[_sl(i)].rearrange("(a b) -> a b", b=1))
            if with_bv:
                BV = pers.tile([1, D], F32, name="bv", tag="bv")
                BVB = pers.tile([P, D], F32, name="bvb", tag="bvb")
                INV64 = pers.tile([P, 1], F32, name="inv64", tag="inv64")
                load(BV[:], bv32[:])
                nc.gpsimd.partition_broadcast(BVB[:], BV[:])
                nc.gpsimd.memset(INV64[:], 1.0 / WSCALE)
            if with_bo:
                BO = pers.tile([1, D], F32, name="bo", tag="bo")
                BOB = pers.tile([P, D], F32, name="bob", tag="bob")
                load(BO[:], bo32[:])
                nc.gpsimd.partition_broadcast(BOB[:], BO[:])

            qT = [pers.tile([P, NQ], F16, name=f"qT{a}", tag=f"qT{a}") for a in range(ND)]
            kT = [pers.tile([P, M], F16, name=f"kT{a}", tag=f"kT{a}") for a in range(ND)]
            # V8[m][p, i, c] = v[key 256m + 2p + i, c] in fp8 (parity split,
            # matching the byte layout the u16-packed transpose produces)
            V8 = [pers.tile([P, 2, D], F8, name=f"v8_{m}", tag=f"v8_{m}") for m in range(NB)]
            # P^T in fp8, transposed as packed uint16 pairs: PTBu[p, m, q]
            # holds bytes (P[q, 256m+2p], P[q, 256m+2p+1]).
            PTBu = pers.tile([P, NB, NQ], mybir.dt.uint16, name="PTBu", tag="PTBu")
            PTB8v = PTBu.bitcast(F8)  # [P, NB, 2*NQ], (q, parity) interleaved
            recip = [pers.tile([P, 1], F32, name=f"recip{t}", tag=f"recip{t}") for t in range(NNT)]
            # OT8[p, ct, n] = O^T[128*ct + p, n] in fp8
            OT8 = pers.tile([P, ND, NQ], F8, name="OT8", tag="OT8")

            # PE warm-up scratch: the PE sits idle ~4us waiting for the
            # first input DMAs, and the clock only ramps to full speed after
            # ~3us of continuous execution. Junk matmuls on memset scratch
            # fill the DMA wait so the real Q projection starts at full clock.
            JW = pers.tile([P, P], F16, name="jw", tag="jw")
            JX = pers.tile([P, 512], F16, name="jx", tag="jx")
            nc.gpsimd.memset(JW[:], 0.0)
            nc.gpsimd.memset(JX[:], 0.0)

            # copy engines for projection PSUM->SBUF rotation
            cengs = [nc.scalar.copy, nc.vector.tensor_copy]

            with tc.tile_pool(name="spool", bufs=3, space="PSUM") as spool, \
                 tc.tile_pool(name="otps", bufs=2, space="PSUM") as otps, \
                 tc.tile_pool(name="ppool", bufs=6) as ppool, \
                 tc.tile_pool(name="stat", bufs=8) as stat, \
                 tc.tile_pool(name="fin", bufs=4) as fin:

                jps = spool.tile([P, M // 2], F32, name="jps", tag="S")
                for _ in range(8):
                    nc.tensor.matmul(jps[:, :512], JW[:], JX[:],
                                     start=True, stop=True)

                # ---- Q projection: groups (c, a), accumulate over d ------------
                ci = [0]

                def proj_group(wtiles, xtiles, dst, btiles, c, a, cw):
                    ps = otps.tile([P, 512], F32, name="ot", tag="ot")
                    for d in range(ND):
                        nc.tensor.matmul(ps[:], wtiles[d][:, _sl(a)],
                                         xtiles[d][:, _sl(c, 512)],
                                         start=(d == 0), stop=(d == ND - 1))
                    if with_bqk:
                        nc.vector.tensor_scalar_add(
                            dst[a][:, _sl(c, 512)], ps[:], btiles[a][:])
                    else:
                        cp = cengs[ci[0] % len(cengs)]
                        ci[0] += 1
                        cp(dst[a][:, _sl(c, 512)], ps[:])

                for c in range(NCH):
                    for a in range(ND):
                        proj_group(WQ, XQT, qT, BQ if with_bqk else None, c, a, NCH)
                for c in range(NMC):
                    for a in range(ND):
                        proj_group(WK, XKVT, kT, BK if with_bqk else None, c, a, NMC)

                # ---- V projection: fp8 DoubleRow over paired d-chunks ----------
                # key = 256b + 2k + two: parity-strided view for the V8 split
                XKV8r = XKV8.rearrange("p d (b k two) -> p d b two k",
                                       b=NB, two=2)

                def v_group(m, i):
                    ps = otps.tile([P, 512], F32, name="ot", tag="ot")
                    for j in range(ND // 2):
                        nc.tensor.matmul(
                            ps[:], XKV8r[:, 2 * j:2 * j + 2, m, i, :],
                            WV8[:, 2 * j:2 * j + 2, :],
                            start=(j == 0), stop=(j == ND // 2 - 1),
                            perf_mode=DR)
                    if with_bv:
                        nc.vector.scalar_tensor_tensor(
                            out=V8[m][:, i, :], in0=ps[:], scalar=INV64[:],
                            in1=BVB[:], op0=ALU.mult, op1=ALU.add)
                    elif i == 0:
                        nc.scalar.mul(V8[m][:, i, :], ps[:], 1.0 / WSCALE)
                    else:
                        # split psum->V8 copies across scalar+vector so the
                        # scalar queue enters the softmax phase drained
                        nc.vector.tensor_scalar_mul(V8[m][:, i, :], ps[:],
                                                    1.0 / WSCALE)

                # ---- scores + softmax ------------------------------------------
                def softmax_tile(t):
                    halves = []
                    nmh = []
                    for h in range(2):
                        sps = spool.tile([P, M // 2], F32, name=f"S{h}", tag="S")
                        for a in range(ND):
                            for c in range(2):
                                mc = h * 2 + c
                                nc.tensor.matmul(sps[:, _sl(c, 512)],
                                                 qT[a][:, _sl(t)],
                                                 kT[a][:, _sl(mc, 512)],
                                                 start=(a == 0), stop=(a == ND - 1))
                        nm = stat.tile([P, 1], F32, name=f"negmax{h}", tag=f"negmax{h}")
                        nc.vector.tensor_reduce(nm[:], sps[:], axis=AX.X,
                                                op=ALU.max, negate=True)
                        halves.append(sps)
                        nmh.append(nm)
                    negmax = stat.tile([P, 1], F32, name="negmax", tag="negmax")
                    nc.vector.tensor_tensor(negmax[:], nmh[0][:], nmh[1][:],
                                            op=ALU.min)
                    # exp writes P directly in fp8; the transpose moves packed
                    # uint16 byte-pairs (fp8 isn't a legal xbar dtype), which
                    # lands adjacent keys in one partition's byte lanes --
                    # exactly the DoubleRow rhs layout PV wants.
                    pt8 = ppool.tile([P, M], F8, name="P", tag="P")
                    rsh = []
                    for h in range(2):
                        # exp -> half-transpose issued immediately so the last
                        # tiles' P^T chain is short (it gates PV).
                        rs = stat.tile([P, 1], F32, name=f"rowsum{h}", tag=f"rowsum{h}")
                        nc.scalar.activation(pt8[:, _sl(h, M // 2)], halves[h][:],
                                             ACTF.Exp, bias=negmax[:], scale=1.0,
                                             accum_out=rs[:])
                        rsh.append(rs)
                        hb = slice(h * (NB // 2), (h + 1) * (NB // 2))
                        nc.sync.dma_start(
                            out=PTBu[:, hb, _sl(t)],
                            in_=pt8[:, _sl(h, M // 2)].bitcast(mybir.dt.uint16),
                            transpose=True)
                    rowsum = stat.tile([P, 1], F32, name="rowsum", tag="rowsum")
                    nc.vector.tensor_tensor(rowsum[:], rsh[0][:], rsh[1][:],
                                            op=ALU.add)
                    rs64 = stat.tile([P, 1], F32, name="rs64", tag="rs64")
                    nc.vector.tensor_scalar_mul(rs64[:], rowsum[:], WSCALE)
                    nc.vector.reciprocal(recip[t][:], rs64[:])

                # V-projection groups interleave between the early score
                # tiles: the PE chews V matmuls while each tile's max->exp
                # chain releases its score psum ring slots (otherwise the
                # ring-3 release latency stalls the PE ~1us per tile).
                for t in range(NNT):
                    softmax_tile(t)
                    if t < 4:
                        for mt in range(4 * t, 4 * t + 4):
                            v_group(mt // 2, mt % 2)

                # ---- PV (fp8 DoubleRow over paired key tiles) ------------------
                def pv_chunk(ck):
                    # ck0's copies ride vector (scalar is still draining the
                    # softmax-tail exps), ck1's ride the then-free scalar.
                    for ct in range(ND):
                        ps = otps.tile([P, 512], F32, name="ot", tag="ot")
                        for m in range(NB):
                            rhs = PTB8v[:, m, :].rearrange(
                                "p (q b) -> p b q", b=2)[:, :, _sl(ck, 512)]
                            nc.tensor.matmul(
                                ps[:], V8[m][:, :, _sl(ct)], rhs,
                                start=(m == 0), stop=(m == NB - 1),
                                perf_mode=DR)
                        nc.scalar.copy(OT8[:, ct, _sl(ck, 512)], ps[:])

                # ---- output projection (fp8 DoubleRow) -------------------------
                def y_tile(t):
                    ps = otps.tile([P, D], F32, name="y", tag="ot")
                    for j in range(ND // 2):
                        nc.tensor.matmul(
                            ps[:], OT8[:, 2 * j:2 * j + 2, _sl(t)],
                            WO8[:, 2 * j:2 * j + 2, :],
                            start=(j == 0), stop=(j == ND // 2 - 1),
                            perf_mode=DR)
                    osb = fin.tile([P, D], F32, name="osb", tag="osb")
                    nc.vector.scalar_tensor_tensor(
                        out=osb[:], in0=ps[:], scalar=recip[t][:],
                        in1=XQ32[t][:], op0=ALU.mult, op1=ALU.add)
                    if with_bo:
                        nc.vector.tensor_add(osb[:], osb[:], BOB[:])
                    nc.sync.dma_start(out=out[_sl(t), :], in_=osb[:])

                pv_chunk(0)
                for t in range(NNT // 2):
                    y_tile(t)
                pv_chunk(1)
                for t in range(NNT // 2, NNT):
                    y_tile(t)

    nc.compile()
    return nc


_BUILD_CACHE = {}


def _get_nc(with_bqk: bool, with_bv: bool, with_bo: bool):
    key = (with_bqk, with_bv, with_bo)
    if key not in _BUILD_CACHE:
        _BUILD_CACHE[key] = _build(with_bqk, with_bv, with_bo)
    return _BUILD_CACHE[key]


def kernel(query, key_value, Wq, bq, Wk, bk, Wv, bv, Wo, bo, _timing=None):
    query = np.asarray(query, dtype=np.float32)
    key_value = np.asarray(key_value, dtype=np.float32)
    Wq = np.asarray(Wq, dtype=np.float32)
    Wk = np.asarray(Wk, dtype=np.float32)
    Wv = np.asarray(Wv, dtype=np.float32)
    Wo = np.asarray(Wo, dtype=np.float32)
    bq = np.asarray(bq, dtype=np.float32)
    bk = np.asarray(bk, dtype=np.float32)
    bv = np.asarray(bv, dtype=np.float32)
    bo = np.asarray(bo, dtype=np.float32)

    with_bqk = bool(np.any(bq)) or bool(np.any(bk))
    with_bv = bool(np.any(bv))
    with_bo = bool(np.any(bo))
    nc = _get_nc(with_bqk, with_bv, with_bo)

    f8 = ml_dtypes.float8_e4m3fn
    wq16 = (Wq * SCALE).astype(np.float16)
    wk16 = Wk.astype(np.float16)
    # [128, 4, 512] chunked layouts for fp8 weights, pre-scaled by 64
    wv8 = np.ascontiguousarray(
        (Wv * WSCALE).astype(f8).reshape(ND, P, D).transpose(1, 0, 2))
    wo8 = np.ascontiguousarray(
        (Wo * WSCALE).astype(f8).reshape(ND, P, D).transpose(1, 0, 2))
    bqs = (bq * SCALE).astype(np.float32)
    bk32 = bk.astype(np.float32)
    bv32 = bv.astype(np.float32).reshape(1, D)
    bo32 = bo.astype(np.float32).reshape(1, D)

    q16 = query.astype(np.float16)
    kv16 = key_value.astype(np.float16)
    kv8 = key_value.astype(f8)

    in_maps = []
    for core in range(N_CORES):
        b, h = divmod(core, 2)
        sl = slice(h * NQ, (h + 1) * NQ)
        im = {
            "xqT16": np.ascontiguousarray(q16[b, sl].T),
            "xkvT16": np.ascontiguousarray(kv16[b].T),
            # [128, 4, 2048]: xkv8[p, j, key] = kv[key, 128j+p]
            "xkv8": np.ascontiguousarray(
                kv8[b].T.reshape(ND, P, M).transpose(1, 0, 2)),
            "xq32": np.ascontiguousarray(query[b, sl]),
            "wq16": wq16, "wk16": wk16, "wv8": wv8, "wo8": wo8,
            "bq": bqs, "bk": bk32, "bv32": bv32, "bo32": bo32,
        }
        in_maps.append(im)

    res = run_bass_kernel_spmd(nc, in_maps, list(range(N_CORES)),
                               **(_timing or {}))
    out = np.empty((B, N, D), dtype=np.float32)
    for core in range(N_CORES):
        b, h = divmod(core, 2)
        out[b, h * NQ:(h + 1) * NQ] = res.results[core]["out"]
    if _timing is not None:
        return out, res
    return out
